# revision 1
# baseline (speedup 1.0000x reference)
"""Trainium2 Bass kernel for nn_DataAugmentation (flip + resized-crop +
brightness/contrast/saturation/hue) — 8-core data-parallel.

Self-contained: takes FULL inputs, shards batch across 8 NeuronCores,
runs one Bass/Tile program per core via run_bass_kernel_spmd, gathers.

v2: f16 pipeline (DVE 2x/4x modes, 4x f16 matmuls), reciprocal-free hue
(out_c = mx - clamp(2cr - ||v-2k*cr|-3cr|, 0, cr), v = esel + cr*(zb+hf6)),
work split across DVE / Pool (gpsimd) / Activation engines, merged DMAs,
f16 output DMA with host-side f32 cast.
"""

import numpy as np

import concourse.bass as bass
import concourse.bacc as bacc
import concourse.tile as tile
import concourse.mybir as mybir
from concourse.bass_utils import run_bass_kernel_spmd
from concourse.dve_spec import (
    Spec, Src0, Src1, C0, C1, C2, Zero, One, maxx, minn, select, Bin, AluOp,
    lower,
)
from concourse.dve_ops import DveOp, DveOpSpec, OPS, CUSTOM_DVE_SPECS, _SUB_OPCODE_FOR_NAME, has_src1

F32 = mybir.dt.float32
F16 = mybir.dt.float16
P = 128
OUT = 64
N_CORES = 8
B_FULL = 4096
B_CORE = B_FULL // N_CORES          # 512
GPAIRS = 16                         # pairs per group
NPAIR = B_CORE // 2                 # 256
NGROUP = NPAIR // GPAIRS            # 16
NFAC = 6                            # bf, cf, sf, osf, cb, hf6
GRAY_W = (0.2989, 0.587, 0.114)


# ---------------------------------------------------------------- custom ops
def _register_op(name, spec):
    if name in _SUB_OPCODE_FOR_NAME:
        for o in OPS:
            if o.name == name:
                return o
    opc = 1 + len(OPS)
    _SUB_OPCODE_FOR_NAME[name] = opc
    shas = {}
    for ver in ("v3", "v4"):
        try:
            s = DveOpSpec(name=name, opcode=opc, uops=lower(spec, ver=ver),
                          rd1_en=has_src1(spec))
            shas[ver] = s.sha(ver)
        except ValueError:
            pass
    op = DveOp(name, spec, subdim=False, uops_sha=shas)
    OPS.append(op)
    CUSTOM_DVE_SPECS[name] = spec
    return op


def _refbc(v, like):
    """Broadcast a [P,1] per-partition scalar (or python float) over `like`."""
    if isinstance(v, np.ndarray) and v.ndim >= 1:
        return v.reshape(v.shape[0], *([1] * (like.ndim - 1))).astype(np.float32)
    return np.float32(v)


def _refsame(v, like):
    """Reshape/broadcast an in1 operand to in0's shape."""
    if v.shape == like.shape:
        return v
    if v.size == like.size:
        return v.reshape(like.shape)
    return np.broadcast_to(v.reshape(v.shape[0], 1, -1) if v.ndim == 2 else v, like.shape)


def _absd(a, b):
    return Bin(AluOp.ABSOLUTE_DIFF, a, b)


# hat(x) = relu(1 - |x - c0|): bilinear interp weight
HAT = _register_op("AUG_HAT", Spec(
    body=maxx(One - _absd(Src0, C0), Zero),
    reference=lambda in0, in1, s0, s1, imm2:
        np.maximum(1.0 - np.abs(in0 - _refbc(s0, in0)), 0.0).astype(np.float32),
))
# g2 = in0*c0 + in1*c1 (grayscale partial)
G2 = _register_op("AUG_G2", Spec(
    body=Src0 * C0 + Src1 * C1,
    reference=lambda in0, in1, s0, s1, imm2:
        (in0 * _refbc(s0, in0) + _refsame(in1, in0) * np.float32(s1)).astype(np.float32),
))
# zb = mr ? 0 : (mg ? c2 : 2*c2)
ZB0 = _register_op("AUG_ZB0", Spec(
    body=select(Src0, Zero, select(Src1, C2, C2 + C2)),
    reference=lambda in0, in1, s0, s1, imm2:
        np.where(in0 != 0, 0.0, np.where(in1 != 0, imm2, 2 * imm2)).astype(np.float32),
))


def _trapq_ref(in0, in1, s0, s1, imm2):
    cr = _refsame(in1, in0).astype(np.float32)
    v = in0.astype(np.float32)
    tri = np.abs(np.abs(v - np.float32(s0) * cr) - np.float32(s1) * cr)
    return np.maximum(cr - np.maximum(tri - cr, 0.0), 0.0).astype(np.float32)


# uc = clamp(2cr - ||v - c0*cr| - c1*cr|, 0, cr)   (cr >= 0)
TRAPQ = _register_op("AUG_TRAPQ", Spec(
    body=maxx(Src1 - maxx(_absd(_absd(Src0, C0 * Src1), C1 * Src1) - Src1,
                          Zero), Zero),
    reference=_trapq_ref,
))
# satcl2 = clamp01(in1 + c0*(in0 - in1))  == clamp01(sf*x + (1-sf)*gray)
SATCL2 = _register_op("AUG_SATCL2", Spec(
    body=minn(maxx(Src1 + C0 * Bin(AluOp.SUBTRACT, Src0, Src1), Zero), One),
    reference=lambda in0, in1, s0, s1, imm2:
        np.clip(_refsame(in1, in0) + _refbc(s0, in0) * (in0 - _refsame(in1, in0)),
                0.0, 1.0).astype(np.float32),
))
# contr = clamp01(c0*min(in0,1) + c1): brightness clamp + contrast affine + clamp
CONTR = _register_op("AUG_CONTR", Spec(
    body=minn(maxx(C0 * minn(Src0, One) + C1, Zero), One),
    reference=lambda in0, in1, s0, s1, imm2:
        np.clip(_refbc(s0, in0) * np.minimum(in0, 1.0) + _refbc(s1, in0),
                0.0, 1.0).astype(np.float32),
))
# g2c = min(in0,1)*c0 + min(in1,1)*c1 (grayscale partial of clamped x1)
G2C = _register_op("AUG_G2C", Spec(
    body=minn(Src0, One) * C0 + minn(Src1, One) * C1,
    reference=lambda in0, in1, s0, s1, imm2:
        (np.minimum(in0, 1.0) * _refbc(s0, in0)
         + np.minimum(_refsame(in1, in0), 1.0) * np.float32(s1)).astype(np.float32),
))
def _esel3_ref(in0, in1, s0, s1, imm2):
    e1 = in0.astype(np.float32)
    e2 = _refsame(in1, in0).astype(np.float32)
    t = e1 + e2
    m = (t <= 0) & (e2 <= 0)
    c = e1 >= 0
    return np.where(m, e1, np.where(c, e2, -t)).astype(np.float32)


def _zbsel_ref(in0, in1, s0, s1, imm2):
    e1 = in0.astype(np.float32)
    e2 = _refsame(in1, in0).astype(np.float32)
    t = e1 + e2
    m = (t <= 0) & (e2 <= 0)
    c = e1 >= 0
    return np.where(m, 0.0, np.where(c, np.float32(imm2), np.float32(2 * imm2))).astype(np.float32)


# esel = e1 if r-max else (e2 if g-max else e3);  e3 == -(e1+e2)
# r-max == (e3>=0 && e2<=0), g-max (on !r) == e1>=0  — exact tie priority
def _mk_esel3():
    e3v = Bin(AluOp.SUBTRACT, Zero - Src0, Src1)
    m = Bin(AluOp.LOGICAL_AND,
            Bin(AluOp.IS_GE, e3v, Zero),
            Bin(AluOp.IS_LE, Src1, Zero))
    return Spec(
        body=select(m, Src0,
                    select(Bin(AluOp.IS_GE, Src0, Zero), Src1, e3v)),
        reference=_esel3_ref,
    )


def _mk_zbsel():
    e3v = Bin(AluOp.SUBTRACT, Zero - Src0, Src1)
    m = Bin(AluOp.LOGICAL_AND,
            Bin(AluOp.IS_GE, e3v, Zero),
            Bin(AluOp.IS_LE, Src1, Zero))
    return Spec(
        body=select(m, Zero,
                    select(Bin(AluOp.IS_GE, Src0, Zero), C0, C1)),
        reference=lambda in0, in1, s0, s1, imm2:
            _zbsel_ref(in0, in1, s0, s1, np.float32(s0)) if False else
            np.where((-(in0 + _refsame(in1, in0)) >= 0) & (_refsame(in1, in0) <= 0), 0.0,
                     np.where(in0 >= 0, np.float32(s0), np.float32(s1))).astype(np.float32),
    )


ESEL3 = _register_op("AUG_ESEL3", _mk_esel3())
ZBSEL = _register_op("AUG_ZBSEL", _mk_zbsel())
# cr = max(|e1|, |e2|, |e1+e2|) == mx - mn
CRE3 = _register_op("AUG_CRE3", Spec(
    body=maxx(maxx(_absd(Src0, Zero), _absd(Src1, Zero)),
              _absd(Src0 + Src1, Zero)),
    reference=lambda in0, in1, s0, s1, imm2:
        np.maximum(np.maximum(np.abs(in0), np.abs(_refsame(in1, in0))),
                   np.abs(in0 + _refsame(in1, in0))).astype(np.float32),
))
# cre1 = max(|in0|, |in1|); cre2 = max(in0, |in1|)
CRE1 = _register_op("AUG_CRE1", Spec(
    body=maxx(maxx(Src0, Zero - Src0), maxx(Src1, Zero - Src1)),
    reference=lambda in0, in1, s0, s1, imm2:
        np.maximum(np.abs(in0), np.abs(_refsame(in1, in0))).astype(np.float32),
))
CRE2 = _register_op("AUG_CRE2", Spec(
    body=maxx(Src0, maxx(Src1, Zero - Src1)),
    reference=lambda in0, in1, s0, s1, imm2:
        np.maximum(in0, np.abs(_refsame(in1, in0))).astype(np.float32),
))


# ---------------------------------------------------------------- device program
def build_nc(b_core=B_CORE, gpairs=GPAIRS, debug=False):
    npair = b_core // 2
    ngroup = npair // gpairs
    assert ngroup * gpairs == npair
    G = gpairs
    FDP = OUT * G          # per-pixel-class free size per group (1024)
    AluT = mybir.AluOpType
    Act = mybir.ActivationFunctionType
    F32R = mybir.dt.float32r
    U8 = mybir.dt.uint8

    nc = bacc.Bacc("TRN2", target_bir_lowering=False, debug=debug)

    x_in = nc.dram_tensor("x_in", [ngroup, 2, OUT, G * 3 * OUT], F32, kind="ExternalInput")
    yx_in = nc.dram_tensor("yx", [b_core, 2, OUT], F32, kind="ExternalInput")
    fac_in = nc.dram_tensor("fac", [ngroup, P, NFAC * G], F32, kind="ExternalInput")
    iota_in = nc.dram_tensor("iota", [P, 2], F32, kind="ExternalInput")
    o2_in = nc.dram_tensor("o2", [P, P], F32, kind="ExternalInput")
    out_d = nc.dram_tensor("out", [ngroup, 2, OUT, G * 3 * OUT], F16, kind="ExternalOutput")

    with tile.TileContext(nc) as tc:
        with tc.tile_pool(name="persist", bufs=1) as pers, \
             tc.tile_pool(name="grp", bufs=2) as grp, \
             tc.tile_pool(name="hue", bufs=2) as hue, \
             tc.tile_pool(name="pp", bufs=2, space="PSUM") as pp:

            IOTA = pers.tile([P, 2], F32)
            O2 = pers.tile([P, P], F32)
            FACALL = pers.tile([P, ngroup * NFAC * G], F32)
            nc.scalar.dma_start(IOTA[:], iota_in[:])

            imgd = [pers.tile([P, 384 * G], F32, tag=f"imgd{i}", name=f"imgd{i}") for i in range(2)]
            posd = [pers.tile([P, 192 * G], F32, tag=f"posd{i}", name=f"posd{i}") for i in range(2)]
            for i, t in enumerate(imgd):
                tr = t[:].rearrange("p (gg c k) -> p gg c k", c=3, k=128)
                eng = nc.vector if i == 0 else nc.gpsimd
                eng.memset(tr[0:64, :, :, 64:128], 0.0)
                eng.memset(tr[64:128, :, :, 0:64], 0.0)
            for i, t in enumerate(posd):
                tr = t[:].rearrange("p (gg m) -> p gg m", m=192)
                eng = nc.vector if i == 0 else nc.gpsimd
                eng.memset(tr[0:64, :, 64:128], 0.0)
                eng.memset(tr[64:128, :, 0:64], 0.0)

            xev = x_in[:].rearrange("g s y (gg c n) -> g s y gg c n", c=3, n=OUT)
            oev = out_d[:]
            yxv = yx_in[:].rearrange("(q s) j k -> q s j k", s=2)

            for g in range(ngroup):
                eo = g & 1
                p0 = g * G
                img = imgd[eo]
                pos = posd[eo]
                imgr = img[:].rearrange("p (gg c k) -> p gg c k", c=3, k=128)
                posr = pos[:].rearrange("p (gg m) -> p gg m", m=192)

                # ---- input DMAs
                for hh in range(2):
                    for s in range(2):
                        r0, r1 = (0, 64) if s == 0 else (64, 128)
                        c0, c1 = (0, 64) if s == 0 else (64, 128)
                        nc.sync.dma_start(
                            imgr[r0:r1, 8 * hh:8 * hh + 8, :, c0:c1],
                            xev[g, s, :, 8 * hh:8 * hh + 8])
                if g == 0:
                    nc.sync.dma_start(O2[:], o2_in[:])
                    nc.sync.dma_start(
                        FACALL[:].rearrange("p (g m) -> p g m", g=ngroup),
                        fac_in[:].transpose([1, 0, 2]))
                for s in range(2):   # A-half rows 0:64 / B-half 64:128
                    r0, r1 = (0, 64) if s == 0 else (64, 128)
                    c0, c1 = (0, 64) if s == 0 else (64, 128)
                    peng = nc.scalar if (g == 0 and s == 0) else nc.gpsimd
                    peng.dma_start(
                        posr[r0:r1, :, c0:c1],
                        yxv[p0:p0 + G, s, 0, :].unsqueeze(0).broadcast_to((64, G, OUT)))
                    peng.dma_start(
                        posr[r0:r1, :, 128:192],
                        yxv[p0:p0 + G, s, 1, :].unsqueeze(0).broadcast_to((64, G, OUT)))

                FAC = FACALL[:, g * NFAC * G:(g + 1) * NFAC * G]
                facr = FAC.rearrange("p (gg s) -> p gg s", s=NFAC)

                # ---- hat weights: relu(1 - |pos - k|) on ACT (2 passes)
                # positions shifted +2 so the zero off-diag blocks give 0.
                posw = grp.tile([P, 192 * G], F32, tag="posw")
                pwr = posw[:].rearrange("p (gg m) -> p gg m", m=192)
                nc.scalar.activation(posw[:], pos[:], Act.Abs, bias=IOTA[:, 1:2])
                nc.scalar.activation(posw[:], posw[:], Act.Relu, bias=1.0, scale=-1.0)
                wr = pwr

                # ---- resize matmuls (fp32r), batched PSUM evicts on ACT
                # (brightness bf is folded into x on the host; resize is linear)
                xbuf = grp.tile([P, 192 * G], F32, tag="xbuf")

                def rs_half(hh):
                    for pb in range(4 * hh, 4 * hh + 4):   # batches of 2 pairs
                        T1 = pp.tile([P, 768], F32, tag="t1", bufs=2)
                        for pi in range(2):
                            p = 2 * pb + pi
                            for c in range(3):
                                nc.tensor.matmul(
                                    T1[:, 384 * pi + 128 * c:384 * pi + 128 * (c + 1)],
                                    imgr[:, p, c, :], wr[:, p, 0:128],
                                    start=True, stop=True)
                        sbt = grp.tile([P, 768], F32, tag="sbt", bufs=3)
                        nc.scalar.copy(sbt[:], T1[:])
                        T2 = pp.tile([P, 384], F32, tag="t2", bufs=2)
                        for pi in range(2):
                            p = 2 * pb + pi
                            off = 192 * pi
                            for c in range(3):
                                nc.tensor.matmul(
                                    T2[:, off + 64 * c:off + 64 * (c + 1)],
                                    sbt[:, 384 * pi + 128 * c:384 * pi + 128 * (c + 1)],
                                    wr[:, p, 128:192], start=True, stop=True)
                        nc.scalar.copy(xbuf[:, 384 * pb:384 * (pb + 1)], T2[:])

                # xbuf holds UNCLAMPED bf*x0r; min(.,1) fused into G2C/TSC/CONTR
                xr = xbuf[:].rearrange("p (gg c k) -> p gg c k", c=3, k=OUT)

                # Half-group chains: emit half-0's ENTIRE DVE chain before
                # any half-1 op (engine queues are in-order). The hue tail is
                # split pre/fin and interleaved across halves so ACT(zbh) and
                # Pool(w,v) round-trips are covered by the other half's work.
                H = G // 2
                HF = OUT * H                        # 512

                gray = grp.tile([P, FDP], F32, tag="gray")
                grayr = gray[:].rearrange("p (gg k) -> p gg k", k=OUT)
                scr = hue.tile([P, FDP], F32, tag="e1")  # e1 doubles as scratch
                mrow = grp.tile([P, 2 * G], F32, tag="mrow")
                Mcol = pp.tile([P, G], F32, tag="mcol", bufs=1)
                tb8 = grp.tile([P, G], F32, tag="tb8")
                e1 = scr
                e2 = hue.tile([P, FDP], F32, tag="e2")
                esel = hue.tile([P, FDP], F32, tag="esel")
                mx = hue.tile([P, FDP], F32, tag="mx")
                cr = hue.tile([P, FDP], F32, tag="cr")
                xout = grp.tile([P, 192 * G], F16, tag="xout")
                xoutr = xout[:].rearrange("p (gg c k) -> p gg c k", c=3, k=OUT)

                def hs(t, h):                       # [P, FDP] half slice
                    return t[:, HF * h:HF * (h + 1)]

                def hgg(t, h):                      # [p, gg, k] half slice
                    return t[:].rearrange("p (gg k) -> p gg k", k=OUT)[:, H * h:H * (h + 1), :]

                def ch(h, c):                       # x channel half slice
                    return xr[:, H * h:H * (h + 1), c, :]

                def chain_A(h):
                    """gray1 -> mean -> contrast -> gray2 -> saturation;
                    queues the Pool channel-diff feeds at the end."""
                    nc.gpsimd.tensor_scalar(hgg(scr, h), ch(h, 2),
                                              1.0, GRAY_W[2], AluT.min, AluT.mult)
                    nc.vector._custom_dve(G2C, out=hgg(gray, h), in0=ch(h, 0),
                                          in1=ch(h, 1), s0=GRAY_W[0], s1=GRAY_W[1])
                    nc.vector.tensor_reduce(mrow[:, H * h:H * (h + 1)],
                                            hgg(gray, h), mybir.AxisListType.X,
                                            AluT.add)
                    nc.vector.tensor_reduce(mrow[:, G + H * h:G + H * (h + 1)],
                                            hgg(scr, h), mybir.AxisListType.X,
                                            AluT.add)
                    nc.vector.tensor_tensor(mrow[:, H * h:H * (h + 1)],
                                            mrow[:, H * h:H * (h + 1)],
                                            mrow[:, G + H * h:G + H * (h + 1)],
                                            AluT.add)
                    nc.tensor.matmul(Mcol[:, H * h:H * (h + 1)], O2[:],
                                     mrow[:, H * h:H * (h + 1)], start=True, stop=True)
                    nc.vector.tensor_tensor(tb8[:, H * h:H * (h + 1)],
                                            Mcol[:, H * h:H * (h + 1)],
                                            facr[:, H * h:H * (h + 1), 4], AluT.mult)
                    for p in range(H * h, H * (h + 1)):
                        nc.vector._custom_dve(
                            CONTR, out=xr[:, p, :, :], in0=xr[:, p, :, :],
                            s0=facr[:, p, 1:2], s1=tb8[:, p:p + 1])

                def chain_A2_g2(h):
                    nc.gpsimd.tensor_scalar(hgg(scr, h), ch(h, 2),
                                            GRAY_W[2], None, AluT.mult)
                    nc.vector._custom_dve(G2, out=hgg(gray, h), in0=ch(h, 0),
                                          in1=ch(h, 1), s0=GRAY_W[0], s1=GRAY_W[1])
                    nc.gpsimd.tensor_tensor(hs(gray, h), hs(gray, h), hs(scr, h),
                                            AluT.add)

                def chain_A2_sat(h):
                    for p in range(H * h, H * (h + 1)):
                        nc.vector._custom_dve(
                            SATCL2, out=xr[:, p, :, :], in0=xr[:, p, :, :],
                            in1=grayr[:, p, :].unsqueeze(1).broadcast_to((P, 3, OUT)),
                            s0=facr[:, p, 2:3])
                    nc.gpsimd.tensor_tensor(hgg(e1, h), ch(h, 1), ch(h, 2),
                                            AluT.subtract)            # e1 = g-b
                    nc.gpsimd.tensor_tensor(hgg(e2, h), ch(h, 2), ch(h, 0),
                                            AluT.subtract)            # e2 = b-r

                def tail_pre(h):
                    """mask-free hue front: mx chain + sign-based ISA ops;
                    ends with ACT zbh and Pool w/v (covered by other half)."""
                    nc.vector.tensor_tensor(hgg(mx, h), ch(h, 0), ch(h, 1), AluT.max)
                    nc.vector.tensor_tensor(hgg(mx, h), hgg(mx, h), ch(h, 2), AluT.max)
                    nc.vector._custom_dve(CRE3, out=hs(cr, h), in0=hs(e1, h),
                                          in1=hs(e2, h))
                    nc.vector._custom_dve(ESEL3, out=hs(esel, h), in0=hs(e1, h),
                                          in1=hs(e2, h))
                    nc.vector._custom_dve(ZBSEL, out=hs(e2, h), in0=hs(e1, h),
                                          in1=hs(e2, h), s0=2.0, s1=4.0)
                    for p in range(H * h, H * (h + 1)):
                        nc.scalar.activation(e2[:, OUT * p:OUT * (p + 1)],
                                             e2[:, OUT * p:OUT * (p + 1)],
                                             Act.Identity, bias=facr[:, p, 5:6])

                def tail_wv(h):
                    nc.vector.tensor_tensor(hs(e1, h), hs(cr, h), hs(e2, h),
                                            AluT.mult)                # w = cr*zbh
                    nc.vector.tensor_tensor(hs(esel, h), hs(esel, h), hs(e1, h),
                                            AluT.add)                 # v = esel+w

                def tail_fin(h):
                    for ci in range(3):
                        tq = e1 if ci != 1 else e2
                        nc.vector._custom_dve(TRAPQ, out=hs(tq, h),
                                              in0=hs(esel, h), in1=hs(cr, h),
                                              s0=float(2 * ci), s1=3.0)
                        nc.gpsimd.tensor_tensor(
                            xoutr[:, H * h:H * (h + 1), ci, :],
                            hgg(mx, h), hgg(tq, h), AluT.subtract)

                rs_half(0)
                chain_A(0)
                rs_half(1)
                chain_A(1)
                chain_A2_g2(0)
                chain_A2_g2(1)
                chain_A2_sat(0)
                chain_A2_sat(1)
                tail_pre(0)
                tail_pre(1)
                tail_wv(0)
                tail_fin(0)
                tail_wv(1)
                tail_fin(1)

                # ---- output DMAs (merged: one per half, contiguous both sides)
                for s in range(2):
                    r0, r1 = (0, 64) if s == 0 else (64, 128)
                    nc.gpsimd.dma_start(oev[g, s], xout[r0:r1, :])

    nc.compile()
    return nc


# ---------------------------------------------------------------- host prep
def host_prep(x, flip_mask, crop_i, crop_j, crop_h, crop_w,
              b_factor, c_factor, s_factor, h_factor,
              b_core=B_CORE, gpairs=GPAIRS):
    f32 = np.float32
    B = x.shape[0]
    npair = b_core // 2
    ngroup = npair // gpairs
    G = gpairs

    ar = (np.arange(OUT, dtype=f32) + f32(0.5))
    ys = crop_i[:, None].astype(f32) + ar[None, :] * (crop_h.astype(f32)[:, None] / f32(OUT)) - f32(0.5)
    xs = crop_j[:, None].astype(f32) + ar[None, :] * (crop_w.astype(f32)[:, None] / f32(OUT)) - f32(0.5)

    def eff(p):
        return np.where(p < 0, p + f32(1.0), np.minimum(p, f32(63.0))).astype(f32)

    ysv = (eff(ys) + f32(2.0)).astype(f32)
    xsv = (np.where(flip_mask[:, None], f32(63.0) - eff(xs), eff(xs)) + f32(2.0)).astype(f32)
    yx = np.stack([ysv, xsv], axis=1)              # [B, 2, 64]

    bf = b_factor.astype(f32)
    cf = c_factor.astype(f32)
    sf = s_factor.astype(f32)
    osf = (f32(1.0) - sf).astype(f32)
    cb = ((f32(1.0) - cf) / f32(4096.0)).astype(f32)
    hf6 = (f32(6.0) * h_factor.astype(f32)).astype(f32)

    kk = np.concatenate([np.arange(64, dtype=f32)] * 2) + f32(2.0)
    iota = np.stack([kk, -kk], axis=1).astype(f32)
    o2 = np.zeros((P, P), dtype=f32)
    o2[:64, :64] = 1.0
    o2[64:, 64:] = 1.0

    # fold brightness into the input: resize is linear, so
    # min(bf*resize(x), 1) == min(resize(bf*x), 1)
    x16 = (x * bf[:, None, None, None]).astype(f32)

    per_core = []
    n_cores = B // b_core
    for k in range(n_cores):
        sl = slice(k * b_core, (k + 1) * b_core)
        # [g, gg, s, c, y, n] -> [g, s, y, gg, c, n]
        xh = x16[sl].reshape(ngroup, G, 2, 3, OUT, OUT).transpose(0, 2, 4, 1, 3, 5)
        xh = np.ascontiguousarray(xh.reshape(ngroup, 2, OUT, G * 3 * OUT))
        fac = np.zeros((ngroup, P, NFAC * G), dtype=f32)
        vals = np.stack([bf[sl], cf[sl], sf[sl], osf[sl], cb[sl], hf6[sl]], -1)  # [b_core, 6]
        vals = vals.reshape(ngroup, G, 2, NFAC)
        for s, rows in ((0, slice(0, 64)), (1, slice(64, 128))):
            v = vals[:, :, s, :].reshape(ngroup, 1, G * NFAC)
            fac[:, rows, :] = np.broadcast_to(v, (ngroup, 64, G * NFAC))
        per_core.append({
            "x_in": xh,
            "yx": np.ascontiguousarray(yx[sl]),
            "fac": np.ascontiguousarray(fac),
            "iota": iota,
            "o2": o2,
        })
    return per_core


_NC_CACHE = {}


def kernel(**inputs):
    x = np.asarray(inputs["x"], dtype=np.float32)
    args = {k: np.asarray(inputs[k]) for k in
            ("flip_mask", "crop_i", "crop_j", "crop_h", "crop_w",
             "b_factor", "c_factor", "s_factor", "h_factor")}
    in_maps = host_prep(x, args["flip_mask"], args["crop_i"], args["crop_j"],
                        args["crop_h"], args["crop_w"], args["b_factor"],
                        args["c_factor"], args["s_factor"], args["h_factor"])
    key = (B_CORE, GPAIRS)
    if key not in _NC_CACHE:
        _NC_CACHE[key] = build_nc(B_CORE, GPAIRS)
    nc = _NC_CACHE[key]
    res = run_bass_kernel_spmd(nc, in_maps, list(range(N_CORES)))
    outs = []
    for r in res.results:
        oh = np.asarray(r["out"]).astype(np.float32)
        # [g, s, y, gg, c, n] -> [g, gg, s, c, y, n] -> [b_core, 3, 64, 64]
        oh = oh.reshape(NGROUP, 2, OUT, GPAIRS, 3, OUT).transpose(0, 3, 1, 4, 2, 5)
        outs.append(oh.reshape(B_CORE, 3, OUT, OUT))
    return np.concatenate(outs, axis=0)


if __name__ == "__main__":
    nc = build_nc()
    print("built ok")



# revision 39
# speedup vs baseline: 1.1571x; 1.1571x over previous
"""Trainium2 Bass kernel for nn_DataAugmentation (flip + resized-crop +
brightness/contrast/saturation/hue) — 8-core data-parallel.

Self-contained: takes FULL inputs, shards batch across 8 NeuronCores,
runs one Bass/Tile program per core via run_bass_kernel_spmd, gathers.

v3: f16 end-to-end (f16 input DMA + f16 matmuls at 1cyc/row, f16 DVE
tensor ops in 2x/4x modes), position broadcast via PE matmul into PSUM
(replaces 1MB/group broadcast DMAs), per-pair accum_out for the contrast
mean (replaces tensor_reduce passes), hue select chain on DVE ISA with
max/diff/bias work rebalanced onto Pool, merged output DMA on SP.
"""

import numpy as np

import concourse.bass as bass
import concourse.bass_isa as bass_isa
import concourse.bacc as bacc
import concourse.tile as tile
import concourse.mybir as mybir
from concourse.bass_utils import run_bass_kernel_spmd
from concourse.dve_spec import (
    Spec, Src0, Src1, C0, C1, C2, Zero, One, maxx, minn, select, Bin, AluOp,
    lower,
)
from concourse.dve_ops import DveOp, DveOpSpec, OPS, CUSTOM_DVE_SPECS, _SUB_OPCODE_FOR_NAME, has_src1
from operator import add as _opadd

F32 = mybir.dt.float32
F16 = mybir.dt.float16
F32R = mybir.dt.float32r
P = 128
OUT = 64
N_CORES = 8
B_FULL = 4096
B_CORE = B_FULL // N_CORES          # 512
GPAIRS = 16                         # pairs per group
NPAIR = B_CORE // 2                 # 256
NGROUP = NPAIR // GPAIRS            # 16
NFAC = 6                            # bf, cf, sf, osf, cb, hf6
GRAY_W = (0.2989, 0.587, 0.114)


# ---------------------------------------------------------------- custom ops
def _register_op(name, spec):
    if name in _SUB_OPCODE_FOR_NAME:
        for o in OPS:
            if o.name == name:
                return o
    opc = 1 + len(OPS)
    _SUB_OPCODE_FOR_NAME[name] = opc
    shas = {}
    for ver in ("v3", "v4"):
        try:
            s = DveOpSpec(name=name, opcode=opc, uops=lower(spec, ver=ver),
                          rd1_en=has_src1(spec))
            shas[ver] = s.sha(ver)
        except ValueError:
            pass
    op = DveOp(name, spec, subdim=False, uops_sha=shas)
    OPS.append(op)
    CUSTOM_DVE_SPECS[name] = spec
    return op


def _refbc(v, like):
    """Broadcast a [P,1] per-partition scalar (or python float) over `like`."""
    if isinstance(v, np.ndarray) and v.ndim >= 1:
        return v.reshape(v.shape[0], *([1] * (like.ndim - 1))).astype(np.float32)
    return np.float32(v)


def _refsame(v, like):
    """Reshape/broadcast an in1 operand to in0's shape."""
    if v.shape == like.shape:
        return v
    if v.size == like.size:
        return v.reshape(like.shape)
    return np.broadcast_to(v.reshape(v.shape[0], 1, -1) if v.ndim == 2 else v, like.shape)


def _absd(a, b):
    return Bin(AluOp.ABSOLUTE_DIFF, a, b)


# hat(x) = relu(1 - |x - c0|): bilinear interp weight
HAT = _register_op("AUG_HAT", Spec(
    body=maxx(One - _absd(Src0, C0), Zero),
    reference=lambda in0, in1, s0, s1, imm2:
        np.maximum(1.0 - np.abs(in0 - _refbc(s0, in0)), 0.0).astype(np.float32),
))
# g2 = in0*c0 + in1*c1 (grayscale partial)
G2 = _register_op("AUG_G2", Spec(
    body=Src0 * C0 + Src1 * C1,
    reference=lambda in0, in1, s0, s1, imm2:
        (in0 * _refbc(s0, in0) + _refsame(in1, in0) * np.float32(s1)).astype(np.float32),
))


def _g2ca_ref(in0, in1, s0, s1, imm2):
    b = (np.minimum(in0, 1.0) * _refbc(s0, in0)
         + np.minimum(_refsame(in1, in0), 1.0) * np.float32(s1)).astype(np.float32)
    return b, b.reshape(b.shape[0], -1).sum(axis=-1, keepdims=True).astype(np.float32)


# g2ca = min(in0,1)*c0 + min(in1,1)*c1; accum_out = row sum (contrast mean)
G2CA = _register_op("AUG_G2CA", Spec(
    body=minn(Src0, One) * C0 + minn(Src1, One) * C1,
    accum=_opadd,
    reference=_g2ca_ref,
))
# g2cm = same, no accum (group-wide gray partial of unclamped x)
G2CM = _register_op("AUG_G2CM", Spec(
    body=minn(Src0, One) * C0 + minn(Src1, One) * C1,
    reference=lambda in0, in1, s0, s1, imm2: _g2ca_ref(in0, in1, s0, s1, imm2)[0],
))


def _g2cb_ref(in0, in1, s0, s1, imm2):
    b = (np.minimum(in0, 1.0) * _refbc(s0, in0)
         + _refsame(in1, in0)).astype(np.float32)
    return b, b.reshape(b.shape[0], -1).sum(axis=-1, keepdims=True).astype(np.float32)


# g2cb = min(in0,1)*c0 + in1; accum_out = row sum (finishes the gray mean)
G2CB = _register_op("AUG_G2CB", Spec(
    body=minn(Src0, One) * C0 + Src1,
    accum=_opadd,
    reference=_g2cb_ref,
))


def _trapq_ref(in0, in1, s0, s1, imm2):
    cr = _refsame(in1, in0).astype(np.float32)
    v = in0.astype(np.float32)
    tri = np.abs(np.abs(v - np.float32(s0) * cr) - np.float32(s1) * cr)
    return np.maximum(cr - np.maximum(tri - cr, 0.0), 0.0).astype(np.float32)


# uc = clamp(2cr - ||v - c0*cr| - c1*cr|, 0, cr)   (cr >= 0)
TRAPQ = _register_op("AUG_TRAPQ", Spec(
    body=maxx(Src1 - maxx(_absd(_absd(Src0, C0 * Src1), C1 * Src1) - Src1,
                          Zero), Zero),
    reference=_trapq_ref,
))
# satcl2 = clamp01(in1 + c0*(in0 - in1))  == clamp01(sf*x + (1-sf)*gray)
SATCL2 = _register_op("AUG_SATCL2", Spec(
    body=minn(maxx(Src1 + C0 * Bin(AluOp.SUBTRACT, Src0, Src1), Zero), One),
    reference=lambda in0, in1, s0, s1, imm2:
        np.clip(_refsame(in1, in0) + _refbc(s0, in0) * (in0 - _refsame(in1, in0)),
                0.0, 1.0).astype(np.float32),
))
# contr = clamp01(c0*min(in0,1) + c1): brightness clamp + contrast affine + clamp
CONTR = _register_op("AUG_CONTR", Spec(
    body=minn(maxx(C0 * minn(Src0, One) + C1, Zero), One),
    reference=lambda in0, in1, s0, s1, imm2:
        np.clip(_refbc(s0, in0) * np.minimum(in0, 1.0) + _refbc(s1, in0),
                0.0, 1.0).astype(np.float32),
))


def _esel3_ref(in0, in1, s0, s1, imm2):
    e1 = in0.astype(np.float32)
    e2 = _refsame(in1, in0).astype(np.float32)
    t = e1 + e2
    m = (t <= 0) & (e2 <= 0)
    c = e1 >= 0
    return np.where(m, e1, np.where(c, e2, -t)).astype(np.float32)


# esel = e1 if r-max else (e2 if g-max else e3);  e3 == -(e1+e2)
# r-max == (e3>=0 && e2<=0), g-max (on !r) == e1>=0  — exact tie priority
def _mk_esel3():
    e3v = Bin(AluOp.SUBTRACT, Zero - Src0, Src1)
    m = Bin(AluOp.LOGICAL_AND,
            Bin(AluOp.IS_GE, e3v, Zero),
            Bin(AluOp.IS_LE, Src1, Zero))
    return Spec(
        body=select(m, Src0,
                    select(Bin(AluOp.IS_GE, Src0, Zero), Src1, e3v)),
        reference=_esel3_ref,
    )


def _mk_zbsel():
    e3v = Bin(AluOp.SUBTRACT, Zero - Src0, Src1)
    m = Bin(AluOp.LOGICAL_AND,
            Bin(AluOp.IS_GE, e3v, Zero),
            Bin(AluOp.IS_LE, Src1, Zero))
    return Spec(
        body=select(m, Zero,
                    select(Bin(AluOp.IS_GE, Src0, Zero), C0, C1)),
        reference=lambda in0, in1, s0, s1, imm2:
            np.where((-(in0 + _refsame(in1, in0)) >= 0) & (_refsame(in1, in0) <= 0), 0.0,
                     np.where(in0 >= 0, np.float32(s0), np.float32(s1))).astype(np.float32),
    )


ESEL3 = _register_op("AUG_ESEL3", _mk_esel3())
ZBSEL = _register_op("AUG_ZBSEL", _mk_zbsel())
# cr = max(|e1|, |e2|, |e1+e2|) == mx - mn
CRE3 = _register_op("AUG_CRE3", Spec(
    body=maxx(maxx(_absd(Src0, Zero), _absd(Src1, Zero)),
              _absd(Src0 + Src1, Zero)),
    reference=lambda in0, in1, s0, s1, imm2:
        np.maximum(np.maximum(np.abs(in0), np.abs(_refsame(in1, in0))),
                   np.abs(in0 + _refsame(in1, in0))).astype(np.float32),
))


# ---------------------------------------------------------------- device program
def build_nc(b_core=B_CORE, gpairs=GPAIRS, debug=False):
    npair = b_core // 2
    ngroup = npair // gpairs
    assert ngroup * gpairs == npair
    G = gpairs
    FDP = OUT * G          # per-pixel-class free size per group (1024)
    PCW = 192 * G          # posc cols / wr cols per group (3072)
    AluT = mybir.AluOpType
    Act = mybir.ActivationFunctionType

    nc = bacc.Bacc("TRN2", target_bir_lowering=False, debug=debug)

    x_in = nc.dram_tensor("x_in", [ngroup, 2, OUT, G * 3 * OUT], F32, kind="ExternalInput")
    posc_in = nc.dram_tensor("posc", [ngroup, 4, PCW], F16, kind="ExternalInput")
    fac_in = nc.dram_tensor("fac", [ngroup, P, NFAC * G], F32, kind="ExternalInput")
    o2_in = nc.dram_tensor("o2", [P, P], F32, kind="ExternalInput")
    iota_in = nc.dram_tensor("iota", [P, 2], F32, kind="ExternalInput")
    sel2_in = nc.dram_tensor("sel2", [4, P], F16, kind="ExternalInput")
    out_d = nc.dram_tensor("out", [ngroup, 2, OUT, G * 3 * OUT], F16, kind="ExternalOutput")

    with tile.TileContext(nc) as tc:
        with tc.tile_pool(name="persist", bufs=1) as pers, \
             tc.tile_pool(name="grp", bufs=2) as grp, \
             tc.tile_pool(name="hue", bufs=2) as hue, \
             tc.tile_pool(name="pp", bufs=2, space="PSUM") as pp:

            IOTA = pers.tile([P, 2], F32)
            SEL2 = pers.tile([4, P], F16)
            O2 = pers.tile([P, P], F32)
            FACALL = pers.tile([P, ngroup * NFAC * G], F32)
            nc.scalar.dma_start(IOTA[:], iota_in[:])
            nc.scalar.dma_start(SEL2[:], sel2_in[:])

            # img layout: [P, (gg, c, 128)] block-diagonal: A image rows 0:64
            # cols 0:64, B rows 64:128 cols 64:128, zero quadrants persistent.
            imgd = [pers.tile([P, 384 * G], F32, tag=f"imgd{i}", name=f"imgd{i}") for i in range(2)]
            for i, t in enumerate(imgd):
                tr = t[:].rearrange("p (gg c k) -> p gg c k", c=3, k=128)
                nc.gpsimd.memset(tr[0:64, :, :, 64:128], 0.0)
                nc.gpsimd.memset(tr[64:128, :, :, 0:64], 0.0)

            xev = x_in[:].rearrange("g s y (gg c n) -> g s y gg c n", c=3, n=OUT)
            oev = out_d[:]

            for g in range(ngroup):
                eo = g & 1
                img = imgd[eo]
                imgr = img[:].rearrange("p (gg c k) -> p gg c k", c=3, k=128)

                # ---- input DMAs
                for hh in range(2):
                    for s in range(2):
                        r0, r1 = (0, 64) if s == 0 else (64, 128)
                        c0, c1 = (0, 64) if s == 0 else (64, 128)
                        nc.sync.dma_start(
                            imgr[r0:r1, 8 * hh:8 * hh + 8, :, c0:c1],
                            xev[g, s, :, 8 * hh:8 * hh + 8])
                pcs = grp.tile([4, PCW], F16, tag="pcs")
                nc.gpsimd.dma_start(pcs[:], posc_in[g])
                if g == 0:
                    nc.sync.dma_start(O2[:], o2_in[:])
                    nc.sync.dma_start(
                        FACALL[:].rearrange("p (g m) -> p g m", g=ngroup),
                        fac_in[:].transpose([1, 0, 2]))

                FAC = FACALL[:, g * NFAC * G:(g + 1) * NFAC * G]
                facr = FAC.rearrange("p (gg s) -> p gg s", s=NFAC)

                # ---- hat weights: PE broadcast of positions into PSUM,
                # then Act: |pos - k| (f16) and relu(1 - t) in place.
                wr_t = grp.tile([P, PCW], F32, tag="posw")
                CH = PCW // 6                         # 512 per chunk (1 bank)
                for j in range(6):
                    posb = pp.tile([P, CH], F32, tag="posb", bufs=1)
                    nc.tensor.matmul(posb[:], SEL2[:],
                                     pcs[:, CH * j:CH * (j + 1)],
                                     start=True, stop=True)
                    wsl = wr_t[:, CH * j:CH * (j + 1)]
                    nc.scalar.activation(wsl, posb[:], Act.Abs, bias=IOTA[:, 1:2])
                    nc.scalar.activation(wsl, wsl, Act.Relu, bias=1.0, scale=-1.0)
                wr = wr_t[:].rearrange("p (gg m) -> p gg m", m=192)

                # ---- resize matmuls (f16), batched PSUM evicts on ACT
                # (brightness bf is folded into x on the host; resize is linear)
                xbuf = grp.tile([P, 192 * G], F32, tag="xbuf")

                def rs_half(hh):
                    for pb in range(4 * hh, 4 * hh + 4):   # batches of 2 pairs
                        T1 = pp.tile([P, 768], F32, tag="t1", bufs=2)
                        for pi in range(2):
                            p = 2 * pb + pi
                            for c in range(3):
                                nc.tensor.matmul(
                                    T1[:, 384 * pi + 128 * c:384 * pi + 128 * (c + 1)],
                                    imgr[:, p, c, :], wr[:, p, 0:128],
                                    start=True, stop=True)
                        sbt = grp.tile([P, 768], F32, tag="sbt", bufs=3)
                        nc.scalar.copy(sbt[:], T1[:])
                        T2 = pp.tile([P, 384], F32, tag="t2", bufs=2)
                        for pi in range(2):
                            p = 2 * pb + pi
                            off = 192 * pi
                            for c in range(3):
                                nc.tensor.matmul(
                                    T2[:, off + 64 * c:off + 64 * (c + 1)],
                                    sbt[:, 384 * pi + 128 * c:384 * pi + 128 * (c + 1)],
                                    wr[:, p, 128:192], start=True, stop=True)
                        nc.scalar.copy(xbuf[:, 384 * pb:384 * (pb + 1)], T2[:])

                # xbuf holds UNCLAMPED bf*x0r (f16); min(.,1) fused in G2CA/CONTR
                xr = xbuf[:].rearrange("p (gg c k) -> p gg c k", c=3, k=OUT)

                # Half-group chains: emit half-0's ENTIRE DVE chain before
                # any half-1 op (engine queues are in-order).
                H = G // 2
                HF = OUT * H                        # 512

                gray = grp.tile([P, FDP], F32, tag="gray")
                grayr = gray[:].rearrange("p (gg k) -> p gg k", k=OUT)
                scr = hue.tile([P, FDP], F32, tag="e1")  # e1 doubles as scratch
                scrr = scr[:].rearrange("p (gg k) -> p gg k", k=OUT)
                m1 = grp.tile([P, G], F32, tag="m1")       # per-pair gray sums
                Mcol = pp.tile([P, G], F32, tag="mcol", bufs=1)
                tb8 = grp.tile([P, G], F32, tag="tb8")
                e1 = scr
                e2 = hue.tile([P, FDP], F32, tag="e2")
                esel = hue.tile([P, FDP], F32, tag="esel")
                mx = hue.tile([P, FDP], F32, tag="mx")
                cr = hue.tile([P, FDP], F32, tag="cr")
                xout = grp.tile([P, 192 * G], F16, tag="xout")
                xoutr = xout[:].rearrange("p (gg c k) -> p gg c k", c=3, k=OUT)

                def hs(t, h):                       # [P, FDP] half slice
                    return t[:, HF * h:HF * (h + 1)]

                def hgg(t, h):                      # [p, gg, k] half slice
                    return t[:].rearrange("p (gg k) -> p gg k", k=OUT)[:, H * h:H * (h + 1), :]

                def ch(h, c):                       # x channel half slice
                    return xr[:, H * h:H * (h + 1), c, :]

                def chain_A(h):
                    """gray partial (group op) + per-pair accum -> mean -> contrast."""
                    hsl = slice(H * h, H * (h + 1))
                    nc.vector._custom_dve(
                        G2CM, out=grayr[:, hsl, :], in0=xr[:, hsl, 0, :],
                        in1=xr[:, hsl, 1, :], s0=GRAY_W[0], s1=GRAY_W[1])
                    for p in range(H * h, H * (h + 1)):
                        nc.vector._custom_dve(
                            G2CB, out=scrr[:, p, :], in0=xr[:, p, 2, :],
                            in1=grayr[:, p, :], s0=GRAY_W[2],
                            accum_out=m1[:, p:p + 1])
                    nc.tensor.matmul(Mcol[:, hsl], O2[:], m1[:, hsl],
                                     start=True, stop=True)
                    nc.vector.tensor_tensor(tb8[:, hsl], Mcol[:, hsl],
                                            facr[:, hsl, 4], AluT.mult)
                    for p in range(H * h, H * (h + 1)):
                        nc.vector._custom_dve(
                            CONTR, out=xr[:, p, :, :], in0=xr[:, p, :, :],
                            s0=facr[:, p, 1:2], s1=tb8[:, p:p + 1])

                def chain_A2_g2(h):
                    nc.gpsimd.tensor_scalar(hgg(scr, h), ch(h, 2),
                                            GRAY_W[2], None, AluT.mult)
                    nc.vector._custom_dve(G2, out=hgg(gray, h), in0=ch(h, 0),
                                          in1=ch(h, 1), s0=GRAY_W[0], s1=GRAY_W[1])
                    nc.gpsimd.tensor_tensor(hs(gray, h), hs(gray, h), hs(scr, h),
                                            AluT.add)

                def chain_A2_sat(h):
                    for p in range(H * h, H * (h + 1)):
                        nc.vector._custom_dve(
                            SATCL2, out=xr[:, p, :, :], in0=xr[:, p, :, :],
                            in1=grayr[:, p, :].unsqueeze(1).broadcast_to((P, 3, OUT)),
                            s0=facr[:, p, 2:3])
                    nc.gpsimd.tensor_tensor(hgg(e1, h), ch(h, 1), ch(h, 2),
                                            AluT.subtract)            # e1 = g-b
                    nc.gpsimd.tensor_tensor(hgg(e2, h), ch(h, 2), ch(h, 0),
                                            AluT.subtract)            # e2 = b-r

                def tail_pre(h):
                    """hue front: mx on Pool, selects on DVE ISA, zbh on Pool."""
                    nc.vector.tensor_tensor(hgg(mx, h), ch(h, 0), ch(h, 1), AluT.max)
                    nc.vector.tensor_tensor(hgg(mx, h), hgg(mx, h), ch(h, 2), AluT.max)
                    nc.vector._custom_dve(CRE3, out=hs(cr, h), in0=hs(e1, h),
                                          in1=hs(e2, h))
                    nc.vector._custom_dve(ESEL3, out=hs(esel, h), in0=hs(e1, h),
                                          in1=hs(e2, h))
                    nc.vector._custom_dve(ZBSEL, out=hs(e2, h), in0=hs(e1, h),
                                          in1=hs(e2, h), s0=2.0, s1=4.0)
                    for p in range(H * h, H * (h + 1)):
                        nc.gpsimd.tensor_scalar(
                            e2[:, OUT * p:OUT * (p + 1)],
                            e2[:, OUT * p:OUT * (p + 1)],
                            facr[:, p, 5:6], None, AluT.add)

                def tail_wv(h):
                    nc.gpsimd.tensor_tensor(hs(e1, h), hs(cr, h), hs(e2, h),
                                            AluT.mult)                # w = cr*zbh
                    nc.gpsimd.tensor_tensor(hs(esel, h), hs(esel, h), hs(e1, h),
                                            AluT.add)                 # v = esel+w

                def tail_fin(h):
                    for ci in range(3):
                        tq = e1 if ci != 1 else e2
                        nc.vector._custom_dve(TRAPQ, out=hs(tq, h),
                                              in0=hs(esel, h), in1=hs(cr, h),
                                              s0=float(2 * ci), s1=3.0)
                        nc.gpsimd.tensor_tensor(
                            xoutr[:, H * h:H * (h + 1), ci, :],
                            hgg(mx, h), hgg(tq, h), AluT.subtract)

                rs_half(0)
                chain_A(0)
                rs_half(1)
                chain_A(1)
                chain_A2_g2(0)
                chain_A2_g2(1)
                chain_A2_sat(0)
                chain_A2_sat(1)
                tail_pre(0)
                tail_pre(1)
                tail_wv(0)
                tail_fin(0)
                tail_wv(1)
                tail_fin(1)

                # ---- output DMA (merged: one per group, contiguous both sides)
                nc.gpsimd.dma_start(
                    oev[g].rearrange("s y m -> (s y) m"), xout[:])

    nc.compile()
    return nc


# ---------------------------------------------------------------- host prep
def host_prep(x, flip_mask, crop_i, crop_j, crop_h, crop_w,
              b_factor, c_factor, s_factor, h_factor,
              b_core=B_CORE, gpairs=GPAIRS):
    f32 = np.float32
    B = x.shape[0]
    npair = b_core // 2
    ngroup = npair // gpairs
    G = gpairs

    ar = (np.arange(OUT, dtype=f32) + f32(0.5))
    ys = crop_i[:, None].astype(f32) + ar[None, :] * (crop_h.astype(f32)[:, None] / f32(OUT)) - f32(0.5)
    xs = crop_j[:, None].astype(f32) + ar[None, :] * (crop_w.astype(f32)[:, None] / f32(OUT)) - f32(0.5)

    def eff(p):
        return np.where(p < 0, p + f32(1.0), np.minimum(p, f32(63.0))).astype(f32)

    ysv = (eff(ys) + f32(2.0)).astype(f32)
    xsv = (np.where(flip_mask[:, None], f32(63.0) - eff(xs), eff(xs)) + f32(2.0)).astype(f32)
    yx = np.stack([ysv, xsv], axis=1)              # [B, 2, 64]

    bf = b_factor.astype(f32)
    cf = c_factor.astype(f32)
    sf = s_factor.astype(f32)
    osf = (f32(1.0) - sf).astype(f32)
    cb = ((f32(1.0) - cf) / f32(4096.0)).astype(f32)
    hf6 = (f32(6.0) * h_factor.astype(f32)).astype(f32)

    kk = np.concatenate([np.arange(64, dtype=f32)] * 2) + f32(2.0)
    iota = np.stack([kk, -kk], axis=1).astype(f32)
    sel2 = np.zeros((4, P), dtype=np.float16)
    sel2[0, :64] = 1.0
    sel2[1, :64] = 1.0
    sel2[2, 64:] = 1.0
    sel2[3, 64:] = 1.0

    # fold brightness into the input: resize is linear, so
    # min(bf*resize(x), 1) == min(resize(bf*x), 1)
    x16 = (x * bf[:, None, None, None]).astype(f32)

    per_core = []
    n_cores = B // b_core
    for k in range(n_cores):
        sl = slice(k * b_core, (k + 1) * b_core)
        # [g, gg, s, c, y, n] -> [g, s, y, gg, c, n]
        xh = x16[sl].reshape(ngroup, G, 2, 3, OUT, OUT).transpose(0, 2, 4, 1, 3, 5)
        xh = np.ascontiguousarray(xh.reshape(ngroup, 2, OUT, G * 3 * OUT))
        # posc[g, r] = per-partition-half pos rows:
        #   r=0 (rows 0:64):  per gg [ ysA(64) | zeros(64) | xsA(64) ]
        #   r=1 (rows 64:128):per gg [ zeros   | ysB       | xsB     ]
        yxc = yx[sl].reshape(ngroup, G, 2, 2, OUT)   # [g, gg, s(pair), yx, 64]
        posc = np.zeros((ngroup, 2, G, 3, OUT), dtype=f32)
        posc[:, 0, :, 0, :] = yxc[:, :, 0, 0, :]
        posc[:, 0, :, 2, :] = yxc[:, :, 0, 1, :]
        posc[:, 1, :, 1, :] = yxc[:, :, 1, 0, :]
        posc[:, 1, :, 2, :] = yxc[:, :, 1, 1, :]
        posc = posc.reshape(ngroup, 2, G * 192)
        phi = posc.astype(np.float16)
        plo = (posc - phi.astype(f32)).astype(np.float16)
        # rows: hi0, lo0, hi1, lo1 — summed by SEL2 in the bcast matmul
        posc = np.ascontiguousarray(
            np.stack([phi[:, 0], plo[:, 0], phi[:, 1], plo[:, 1]], axis=1))
        fac = np.zeros((ngroup, P, NFAC * G), dtype=f32)
        vals = np.stack([bf[sl], cf[sl], sf[sl], osf[sl], cb[sl], hf6[sl]], -1)  # [b_core, 6]
        vals = vals.reshape(ngroup, G, 2, NFAC)
        for s, rows in ((0, slice(0, 64)), (1, slice(64, 128))):
            v = vals[:, :, s, :].reshape(ngroup, 1, G * NFAC)
            fac[:, rows, :] = np.broadcast_to(v, (ngroup, 64, G * NFAC))
        o2 = np.zeros((P, P), dtype=f32)
        o2[:64, :64] = 1.0
        o2[64:, 64:] = 1.0
        per_core.append({
            "x_in": xh,
            "o2": o2,
            "posc": posc,
            "fac": np.ascontiguousarray(fac),
            "iota": iota,
            "sel2": sel2,
        })
    return per_core


_NC_CACHE = {}


def kernel(**inputs):
    x = np.asarray(inputs["x"], dtype=np.float32)
    args = {k: np.asarray(inputs[k]) for k in
            ("flip_mask", "crop_i", "crop_j", "crop_h", "crop_w",
             "b_factor", "c_factor", "s_factor", "h_factor")}
    in_maps = host_prep(x, args["flip_mask"], args["crop_i"], args["crop_j"],
                        args["crop_h"], args["crop_w"], args["b_factor"],
                        args["c_factor"], args["s_factor"], args["h_factor"])
    key = (B_CORE, GPAIRS)
    if key not in _NC_CACHE:
        _NC_CACHE[key] = build_nc(B_CORE, GPAIRS)
    nc = _NC_CACHE[key]
    res = run_bass_kernel_spmd(nc, in_maps, list(range(N_CORES)))
    outs = []
    for r in res.results:
        oh = np.asarray(r["out"]).astype(np.float32)
        # [g, s, y, gg, c, n] -> [g, gg, s, c, y, n] -> [b_core, 3, 64, 64]
        oh = oh.reshape(NGROUP, 2, OUT, GPAIRS, 3, OUT).transpose(0, 3, 1, 4, 2, 5)
        outs.append(oh.reshape(B_CORE, 3, OUT, OUT))
    return np.concatenate(outs, axis=0)


if __name__ == "__main__":
    nc = build_nc()
    print("built ok")
